# revision 24
# baseline (speedup 1.0000x reference)
"""Trainium2 Bass kernel for AttnSum3d pooling.

Math (per batch):
    xm = input * mask[:, None]                      # [L, D]
    S  = xm @ xm.T                                  # [L, L], symmetric
    w  = softmax(S, axis=0)  (columns sum to 1 over rows)
    out[d]       = (1/L) * sum_m sum_l w[l, m] xm[l, d]
    attn_mean[m] = (1/L) * sum_l w[l, m] = 1/L      (constant!)

Kernel computes, per m-block of 128 columns (stored row-wise thanks to
symmetry: S_j[m, l] for m in block j):
    c[m]      = sqrt(nsq[m] * max_l nsq[l]) >= max_l S[l, m]   (Cauchy-Schwarz)
    P_j[m, l] = exp(S_j[m, l] - c[m])        (ACT, bias=-c, accum_out=colsum)
    v[m]      = 1 / colsum[m]
    r[l]     += sum_m v[m] * P_j[m, l]       (TensorE, lhsT=v)
then out = (1/L) * r @ xm via 16 accumulated [128,1]x[128,128] matmuls.

Data-parallel over batch: 16 batches -> 8 cores x 2 batches.
"""

import sys

for _p in ("/opt/trn_rl_repo",):
    if _p not in sys.path:
        sys.path.insert(0, _p)

import numpy as np

B, L, D = 16, 2048, 128
NCORES = 8
BPC = B // NCORES          # batches per core
NT = L // 128              # 16 tiles of 128 along L
TPB = L // 128             # elements of L per partition in the (p t) layout

_CACHE = {}


def _build_nc(stage=4, batches=BPC):
    import concourse.bacc as bacc
    import concourse.tile as tile
    from concourse import mybir
    from concourse.masks import make_identity

    f32 = mybir.dt.float32
    f32r = mybir.dt.float32r
    bf16 = mybir.dt.bfloat16
    AF = mybir.ActivationFunctionType
    ALU = mybir.AluOpType
    AX = mybir.AxisListType

    nc = bacc.Bacc("TRN2", target_bir_lowering=False, debug=False)

    x_d = nc.dram_tensor("input", [BPC, L, D], f32, kind="ExternalInput").ap()
    m_d = nc.dram_tensor("mask", [BPC, L], f32, kind="ExternalInput").ap()
    o_d = nc.dram_tensor("out", [BPC, D], f32, kind="ExternalOutput").ap()

    with tile.TileContext(nc) as tc:
        with (
            tc.tile_pool(name="consts", bufs=1) as consts,
            tc.tile_pool(name="xb", bufs=2) as xb,
            tc.tile_pool(name="pb", bufs=4) as pb,
            tc.tile_pool(name="small", bufs=4) as small,
            tc.tile_pool(name="psS", bufs=2, space="PSUM") as psS,
            tc.tile_pool(name="psR", bufs=1, space="PSUM") as psR,
        ):
            identity = consts.tile([128, 128], f32)
            make_identity(nc, identity)
            ones_row = consts.tile([1, 128], f32)
            nc.vector.memset(ones_row, 1.0)

            for b in range(batches):
                # ---------------- load ----------------
                # partition p holds rows l = TPB*p + t  (16 contiguous rows
                # = 8KB per partition -> single fat DMA descriptor each)
                xp = xb.tile([128, TPB, D], f32, name=f"xp{b}", tag="xp")
                xsrc = x_d[b].rearrange("(p t) d -> p t d", p=128)
                for c in range(8):  # split across DMA queues
                    nc.sync.dma_start(
                        out=xp[:, 2 * c : 2 * c + 2, :], in_=xsrc[:, 2 * c : 2 * c + 2, :]
                    )
                mask_sb = xb.tile([128, TPB], f32, name=f"mask{b}", tag="mask")
                nc.sync.dma_start(
                    out=mask_sb[:], in_=m_d[b].rearrange("(p t) -> p t", p=128)
                )

                # ---------------- xm = x*mask, nsq = |xm|^2 ----------------
                xm = xb.tile([128, NT, D], f32, name=f"xm{b}", tag="xm")
                nsq = xb.tile([128, NT], f32, name=f"nsq{b}", tag="nsq")
                sq = xb.tile([128, D], f32, name=f"sq{b}", tag="sq")
                for t in range(NT):
                    nc.vector.tensor_scalar_mul(
                        xm[:, t, :], xp[:, t, :], mask_sb[:, t : t + 1]
                    )
                for t in range(NT):
                    nc.vector.tensor_mul(sq[:], xm[:, t, :], xm[:, t, :])
                    nc.vector.reduce_sum(nsq[:, t : t + 1], sq[:], AX.X)

                # ---------------- N2max = max_l nsq[l], broadcast ----------
                nmaxp = xb.tile([128, 1], f32, name=f"nmaxp{b}", tag="nmaxp")
                nc.vector.reduce_max(nmaxp[:], nsq[:], AX.X)
                tps = psS.tile([1, 128], f32, name=f"tpn{b}", tag="ps")
                nc.tensor.transpose(tps[:], nmaxp[:], identity[:])
                nmax_row = xb.tile([1, 128], f32, name=f"nmr{b}", tag="nmr")
                nc.vector.tensor_copy(nmax_row[:], tps[:])
                n2max = xb.tile([1, 1], f32, name=f"n2max{b}", tag="n2max")
                nc.vector.reduce_max(n2max[:], nmax_row[:], AX.X)
                bps = psR.tile([128, 1], f32, name=f"bps{b}", tag="r")
                nc.tensor.matmul(bps[:], ones_row[:], n2max[:], start=True, stop=True)
                n2b = xb.tile([128, 1], f32, name=f"n2b{b}", tag="n2b")
                nc.vector.tensor_copy(n2b[:], bps[:])

                # ---------------- c ~ sqrt(nsq*N2max), all on DVE ----------
                # exponent-halving sqrt approximation (+/-3.5%), scaled by
                # 1.06 so c >= true column max; keeps ACT exp-table resident
                zt = xb.tile([128, NT], f32, name=f"zt{b}", tag="zt")
                nc.vector.tensor_scalar_mul(zt[:], nsq[:], n2b[:, 0:1])
                zi = zt[:].bitcast(mybir.dt.int32)
                nc.vector.tensor_scalar(
                    zi, zi, 1, None, op0=ALU.arith_shift_right
                )
                nc.vector.tensor_scalar(
                    zi, zi, 0x1FC00000, None, op0=ALU.add
                )
                negc = xb.tile([128, NT], f32, name=f"negc{b}", tag="negc")
                nc.vector.tensor_scalar_mul(negc[:], zt[:], -1.06)

                # ---------------- xmT (bf16) via TensorE transpose ---------
                xmT = xb.tile([128, L], bf16, name=f"xmT{b}", tag="xmT")
                for t in range(NT):
                    tp = psS.tile([128, 128], f32, name=f"tp{b}_{t}", tag="ps")
                    nc.tensor.transpose(tp[:], xm[:, t, :], identity[:])
                    nc.vector.tensor_copy(xmT[:, t * 128 : (t + 1) * 128], tp[:])

                if stage <= 1:
                    o_sb1 = xb.tile([1, D], f32, name=f"o_sb{b}", tag="o_sb")
                    nc.vector.memset(o_sb1[:], 0.5)
                    nc.vector.tensor_copy(o_sb1[0:1, 0:1], negc[0:1, 0:1])
                    nc.sync.dma_start(out=o_d[b : b + 1, :], in_=o_sb1[:])
                    continue

                # ---------------- main loop over m-blocks ------------------
                if stage >= 3:
                    r_ps = psR.tile([1, L], f32, name=f"r_ps{b}", tag="r")
                for jb in range(NT):
                    lhsT = xmT[:, jb * 128 : (jb + 1) * 128]
                    csum = small.tile([128, 2], f32, name=f"cs{b}_{jb}", tag="cs")
                    Ph = []
                    for h in range(2):
                        S_ps = psS.tile(
                            [128, 1024], f32, name=f"S{b}_{jb}_{h}", tag="ps"
                        )
                        for k in range(2):
                            nc.tensor.matmul(
                                S_ps[:, k * 512 : (k + 1) * 512],
                                lhsT,
                                xmT[
                                    :, h * 1024 + k * 512 : h * 1024 + (k + 1) * 512
                                ],
                                start=True,
                                stop=True,
                            )
                        P = pb.tile([128, 1024], bf16, name=f"P{b}_{jb}_{h}", tag="P")
                        nc.scalar.activation(
                            P[:],
                            S_ps[:],
                            AF.Exp,
                            bias=negc[:, jb : jb + 1],
                            scale=1.0,
                            accum_out=csum[:, h : h + 1],
                        )
                        Ph.append(P)

                    cst = small.tile([128, 1], f32, name=f"cst{b}_{jb}", tag="cst")
                    nc.vector.tensor_add(cst[:], csum[:, 0:1], csum[:, 1:2])
                    vj = small.tile([128, 1], f32, name=f"vj{b}_{jb}", tag="vj")
                    nc.vector.reciprocal(vj[:], cst[:])
                    vjb = small.tile([128, 1], bf16, name=f"vjb{b}_{jb}", tag="vjb")
                    nc.vector.tensor_scalar_mul(vjb[:], vj[:], 1.0 / L)

                    if stage >= 3:
                        for h in range(2):
                            for k in range(2):
                                nc.tensor.matmul(
                                    r_ps[
                                        0:1,
                                        h * 1024 + k * 512 : h * 1024 + (k + 1) * 512,
                                    ],
                                    vjb[:],
                                    Ph[h][:, k * 512 : (k + 1) * 512],
                                    start=(jb == 0),
                                    stop=(jb == NT - 1),
                                )

                if stage <= 2:
                    o_sb2 = xb.tile([1, D], f32, name=f"o_sb{b}", tag="o_sb")
                    nc.vector.memset(o_sb2[:], 0.5)
                    nc.vector.tensor_copy(o_sb2[0:1, 0:1], vjb[0:1, 0:1])
                    nc.sync.dma_start(out=o_d[b : b + 1, :], in_=o_sb2[:])
                    continue

                # ---------------- out = (1/L) * r @ xm ---------------------
                r_sb = xb.tile([1, L], f32, name=f"r_sb{b}", tag="r_sb")
                nc.vector.tensor_copy(r_sb[:], r_ps[:])
                if stage <= 3:
                    o_sb3 = xb.tile([1, D], f32, name=f"o_sb{b}", tag="o_sb")
                    nc.vector.tensor_copy(o_sb3[:], r_sb[0:1, 0:D])
                    nc.sync.dma_start(out=o_d[b : b + 1, :], in_=o_sb3[:])
                    continue
                rT = xb.tile([128, NT], f32, name=f"rT{b}", tag="rT")
                for i in range(NT):
                    tpr = psS.tile([128, 1], f32, name=f"tpr{b}_{i}", tag="ps")
                    nc.tensor.transpose(
                        tpr[:], r_sb[0:1, i * 128 : (i + 1) * 128], identity[0:1, 0:1]
                    )
                    nc.vector.tensor_copy(rT[:, i : i + 1], tpr[:])

                o_ps = psR.tile([1, D], f32, name=f"o_ps{b}", tag="r")
                for i in range(NT):
                    nc.tensor.matmul(
                        o_ps[:],
                        rT[:, i : i + 1],
                        xm[:, i, :],
                        start=(i == 0),
                        stop=(i == NT - 1),
                    )
                o_sb = xb.tile([1, D], f32, name=f"o_sb{b}", tag="o_sb")
                nc.vector.tensor_copy(o_sb[:], o_ps[:])
                nc.sync.dma_start(out=o_d[b : b + 1, :], in_=o_sb[:])

    nc.compile()
    return nc


def _get_nc():
    import os

    stage = int(os.environ.get("K_STAGE", "4"))
    batches = int(os.environ.get("K_BATCHES", str(BPC)))
    key = ("nc", stage, batches)
    if key not in _CACHE:
        _CACHE[key] = _build_nc(stage=stage, batches=batches)
    return _CACHE[key]


def _in_maps(inputs):
    x = np.ascontiguousarray(np.asarray(inputs["input"], dtype=np.float32))
    m = np.ascontiguousarray(np.asarray(inputs["mask"], dtype=np.float32))
    assert x.shape == (B, L, D) and m.shape == (B, L)
    return [
        {
            "input": np.ascontiguousarray(x[c * BPC : (c + 1) * BPC]),
            "mask": np.ascontiguousarray(m[c * BPC : (c + 1) * BPC]),
        }
        for c in range(NCORES)
    ]


def _enable_tracing():
    """Shim antenv.axon_hooks (absent in this container) so
    run_bass_kernel_spmd(trace=True) can capture NTFF profiles through
    the axon .so, and neutralize the S3 artifact upload."""
    if _CACHE.get("trace_shim"):
        return
    import types

    import antenv

    if not hasattr(antenv, "axon_hooks"):
        mod = types.ModuleType("antenv.axon_hooks")
        mod._hook = None

        def set_axon_ntff_profile_hook(h):
            mod._hook = h

        def get_axon_ntff_profile_hook():
            return mod._hook

        mod.set_axon_ntff_profile_hook = set_axon_ntff_profile_hook
        mod.get_axon_ntff_profile_hook = get_axon_ntff_profile_hook
        sys.modules["antenv.axon_hooks"] = mod
        antenv.axon_hooks = mod

    from antenv.axon_hooks import get_axon_ntff_profile_hook, set_axon_ntff_profile_hook

    if get_axon_ntff_profile_hook() is None:
        if "/root/.axon_site" not in sys.path:
            sys.path.insert(0, "/root/.axon_site")
        from trn_agent_boot.trn_boot import _ntff_profile_via_ctypes

        set_axon_ntff_profile_hook(
            _ntff_profile_via_ctypes("/opt/axon/libaxon_pjrt.so")
        )

    import concourse.bass_utils as bu

    bu.upload_artifacts = lambda tmpdir: f"local://{tmpdir}"
    _CACHE["trace_shim"] = True


def _run(inputs, trace=False, **kw):
    from concourse.bass_utils import run_bass_kernel_spmd

    if trace:
        _enable_tracing()
    nc = _get_nc()
    res = run_bass_kernel_spmd(
        nc, _in_maps(inputs), core_ids=list(range(NCORES)), trace=trace, **kw
    )
    outs = np.stack([res.results[c]["out"] for c in range(NCORES)])  # [8, BPC, D]
    out_full = outs.reshape(B, 1, D).astype(np.float32)
    attn_mean = np.full((B, L), 1.0 / L, dtype=np.float32)
    return (out_full, attn_mean), res


def kernel(**inputs):
    (out_full, attn_mean), _ = _run(inputs, trace=False)
    return (out_full, attn_mean)


# revision 27
# speedup vs baseline: 1.0423x; 1.0423x over previous
"""Trainium2 Bass kernel for AttnSum3d pooling.

Math (per batch):
    xm = input * mask[:, None]                      # [L, D]
    S  = xm @ xm.T                                  # [L, L], symmetric
    w  = softmax(S, axis=0)  (columns sum to 1 over rows)
    out[d]       = (1/L) * sum_m sum_l w[l, m] xm[l, d]
    attn_mean[m] = (1/L) * sum_l w[l, m] = 1/L      (constant!)

Kernel computes, per m-block of 128 columns (stored row-wise thanks to
symmetry: S_j[m, l] for m in block j):
    c[m]      = sqrt(nsq[m] * max_l nsq[l]) >= max_l S[l, m]   (Cauchy-Schwarz)
    P_j[m, l] = exp(S_j[m, l] - c[m])        (ACT, bias=-c, accum_out=colsum)
    v[m]      = 1 / colsum[m]
    r[l]     += sum_m v[m] * P_j[m, l]       (TensorE, lhsT=v)
then out = (1/L) * r @ xm via 16 accumulated [128,1]x[128,128] matmuls.

Data-parallel over batch: 16 batches -> 8 cores x 2 batches.
"""

import sys

for _p in ("/opt/trn_rl_repo",):
    if _p not in sys.path:
        sys.path.insert(0, _p)

import numpy as np

B, L, D = 16, 2048, 128
NCORES = 8
BPC = B // NCORES          # batches per core
NT = L // 128              # 16 tiles of 128 along L
TPB = L // 128             # elements of L per partition in the (p t) layout

_CACHE = {}


def _build_nc(stage=4, batches=BPC):
    import concourse.bacc as bacc
    import concourse.tile as tile
    from concourse import mybir
    from concourse.masks import make_identity

    f32 = mybir.dt.float32
    f32r = mybir.dt.float32r
    bf16 = mybir.dt.bfloat16
    AF = mybir.ActivationFunctionType
    ALU = mybir.AluOpType
    AX = mybir.AxisListType

    nc = bacc.Bacc("TRN2", target_bir_lowering=False, debug=False)

    x_d = nc.dram_tensor("input", [BPC, L, D], f32, kind="ExternalInput").ap()
    m_d = nc.dram_tensor("mask", [BPC, L], f32, kind="ExternalInput").ap()
    o_d = nc.dram_tensor("out", [BPC, D], f32, kind="ExternalOutput").ap()

    with tile.TileContext(nc) as tc:
        with (
            tc.tile_pool(name="consts", bufs=1) as consts,
            tc.tile_pool(name="xb", bufs=2) as xb,
            tc.tile_pool(name="pb", bufs=4) as pb,
            tc.tile_pool(name="small", bufs=4) as small,
            tc.tile_pool(name="psS", bufs=2, space="PSUM") as psS,
            tc.tile_pool(name="psR", bufs=1, space="PSUM") as psR,
        ):
            identity = consts.tile([128, 128], f32)
            make_identity(nc, identity)
            ones_row = consts.tile([1, 128], f32)
            nc.vector.memset(ones_row, 1.0)

            for b in range(batches):
                # ---------------- load ----------------
                # partition p holds rows l = TPB*p + t  (16 contiguous rows
                # = 8KB per partition -> single fat DMA descriptor each)
                xp = xb.tile([128, TPB, D], f32, name=f"xp{b}", tag="xp")
                xsrc = x_d[b].rearrange("(p t) d -> p t d", p=128)
                for c in range(2):
                    nc.sync.dma_start(
                        out=xp[:, 8 * c : 8 * c + 8, :], in_=xsrc[:, 8 * c : 8 * c + 8, :]
                    )
                mask_sb = xb.tile([128, TPB], f32, name=f"mask{b}", tag="mask")
                nc.sync.dma_start(
                    out=mask_sb[:], in_=m_d[b].rearrange("(p t) -> p t", p=128)
                )

                # ---------------- xm = x*mask, nsq = |xm|^2 ----------------
                xm = xb.tile([128, NT, D], f32, name=f"xm{b}", tag="xm")
                nsq = xb.tile([128, NT], f32, name=f"nsq{b}", tag="nsq")
                for t in range(NT):
                    nc.vector.tensor_scalar_mul(
                        xm[:, t, :], xp[:, t, :], mask_sb[:, t : t + 1]
                    )
                # nsq[l] = sum_d (mask*x)^2 on the (otherwise idle) ACT
                # engine: Square is present in every table set, and this
                # pre-warms the exp set before the main loop needs it
                for t in range(NT):
                    sqj = small.tile([128, D], f32, name=f"sqj{b}_{t}", tag="sqj")
                    nc.scalar.activation(
                        sqj[:],
                        xp[:, t, :],
                        AF.Square,
                        scale=mask_sb[:, t : t + 1],
                        accum_out=nsq[:, t : t + 1],
                    )

                # ---------------- N2max = max_l nsq[l], broadcast ----------
                nmaxp = xb.tile([128, 1], f32, name=f"nmaxp{b}", tag="nmaxp")
                nc.vector.reduce_max(nmaxp[:], nsq[:], AX.X)
                tps = psS.tile([1, 128], f32, name=f"tpn{b}", tag="ps")
                nc.tensor.transpose(tps[:], nmaxp[:], identity[:])
                nmax_row = xb.tile([1, 128], f32, name=f"nmr{b}", tag="nmr")
                nc.vector.tensor_copy(nmax_row[:], tps[:])
                n2max = xb.tile([1, 1], f32, name=f"n2max{b}", tag="n2max")
                nc.vector.reduce_max(n2max[:], nmax_row[:], AX.X)
                bps = psS.tile([128, 1], f32, name=f"bps{b}", tag="ps")
                nc.tensor.matmul(bps[:], ones_row[:], n2max[:], start=True, stop=True)
                n2b = xb.tile([128, 1], f32, name=f"n2b{b}", tag="n2b")
                nc.vector.tensor_copy(n2b[:], bps[:])

                # ---------------- c ~ sqrt(nsq*N2max), all on DVE ----------
                # exponent-halving sqrt approximation (+/-3.5%), scaled by
                # 1.06 so c >= true column max; keeps ACT exp-table resident
                zt = xb.tile([128, NT], f32, name=f"zt{b}", tag="zt")
                nc.vector.tensor_scalar_mul(zt[:], nsq[:], n2b[:, 0:1])
                zi = zt[:].bitcast(mybir.dt.int32)
                nc.vector.tensor_scalar(
                    zi, zi, 1, None, op0=ALU.arith_shift_right
                )
                nc.vector.tensor_scalar(
                    zi, zi, 0x1FC00000, None, op0=ALU.add
                )
                negc = xb.tile([128, NT], f32, name=f"negc{b}", tag="negc")
                nc.vector.tensor_scalar_mul(negc[:], zt[:], -1.06)

                # ---------------- xmT (bf16) via TensorE transpose ---------
                xmT = xb.tile([128, L], bf16, name=f"xmT{b}", tag="xmT")
                for t in range(NT):
                    tp = psS.tile([128, 128], f32, name=f"tp{b}_{t}", tag="ps")
                    nc.tensor.transpose(tp[:], xm[:, t, :], identity[:])
                    nc.vector.tensor_copy(xmT[:, t * 128 : (t + 1) * 128], tp[:])

                if stage <= 1:
                    o_sb1 = xb.tile([1, D], f32, name=f"o_sb{b}", tag="o_sb")
                    nc.vector.memset(o_sb1[:], 0.5)
                    nc.vector.tensor_copy(o_sb1[0:1, 0:1], negc[0:1, 0:1])
                    nc.sync.dma_start(out=o_d[b : b + 1, :], in_=o_sb1[:])
                    continue

                # ---------------- main loop over m-blocks ------------------
                if stage >= 3:
                    r_ps = psR.tile([1, L], f32, name=f"r_ps{b}", tag="r")
                for jb in range(NT):
                    lhsT = xmT[:, jb * 128 : (jb + 1) * 128]
                    csum = small.tile([128, 2], f32, name=f"cs{b}_{jb}", tag="cs")
                    Ph = []
                    for h in range(2):
                        S_ps = psS.tile(
                            [128, 1024], f32, name=f"S{b}_{jb}_{h}", tag="ps"
                        )
                        for k in range(2):
                            nc.tensor.matmul(
                                S_ps[:, k * 512 : (k + 1) * 512],
                                lhsT,
                                xmT[
                                    :, h * 1024 + k * 512 : h * 1024 + (k + 1) * 512
                                ],
                                start=True,
                                stop=True,
                            )
                        P = pb.tile([128, 1024], bf16, name=f"P{b}_{jb}_{h}", tag="P")
                        nc.scalar.activation(
                            P[:],
                            S_ps[:],
                            AF.Exp,
                            bias=negc[:, jb : jb + 1],
                            scale=1.0,
                            accum_out=csum[:, h : h + 1],
                        )
                        Ph.append(P)

                    cst = small.tile([128, 1], f32, name=f"cst{b}_{jb}", tag="cst")
                    nc.vector.tensor_add(cst[:], csum[:, 0:1], csum[:, 1:2])
                    vj = small.tile([128, 1], f32, name=f"vj{b}_{jb}", tag="vj")
                    nc.vector.reciprocal(vj[:], cst[:])
                    vjb = small.tile([128, 1], bf16, name=f"vjb{b}_{jb}", tag="vjb")
                    nc.vector.tensor_scalar_mul(vjb[:], vj[:], 1.0 / L)

                    if stage >= 3:
                        for h in range(2):
                            for k in range(2):
                                nc.tensor.matmul(
                                    r_ps[
                                        0:1,
                                        h * 1024 + k * 512 : h * 1024 + (k + 1) * 512,
                                    ],
                                    vjb[:],
                                    Ph[h][:, k * 512 : (k + 1) * 512],
                                    start=(jb == 0),
                                    stop=(jb == NT - 1),
                                )

                if stage <= 2:
                    o_sb2 = xb.tile([1, D], f32, name=f"o_sb{b}", tag="o_sb")
                    nc.vector.memset(o_sb2[:], 0.5)
                    nc.vector.tensor_copy(o_sb2[0:1, 0:1], vjb[0:1, 0:1])
                    nc.sync.dma_start(out=o_d[b : b + 1, :], in_=o_sb2[:])
                    continue

                # ---------------- out = (1/L) * r @ xm ---------------------
                r_sb = xb.tile([1, L], f32, name=f"r_sb{b}", tag="r_sb")
                nc.vector.tensor_copy(r_sb[:], r_ps[:])
                if stage <= 3:
                    o_sb3 = xb.tile([1, D], f32, name=f"o_sb{b}", tag="o_sb")
                    nc.vector.tensor_copy(o_sb3[:], r_sb[0:1, 0:D])
                    nc.sync.dma_start(out=o_d[b : b + 1, :], in_=o_sb3[:])
                    continue
                rT = xb.tile([128, NT], f32, name=f"rT{b}", tag="rT")
                for i in range(NT):
                    tpr = psS.tile([128, 1], f32, name=f"tpr{b}_{i}", tag="ps")
                    nc.tensor.transpose(
                        tpr[:], r_sb[0:1, i * 128 : (i + 1) * 128], identity[0:1, 0:1]
                    )
                    nc.vector.tensor_copy(rT[:, i : i + 1], tpr[:])

                o_ps = psR.tile([1, D], f32, name=f"o_ps{b}", tag="r")
                for i in range(NT):
                    nc.tensor.matmul(
                        o_ps[:],
                        rT[:, i : i + 1],
                        xm[:, i, :],
                        start=(i == 0),
                        stop=(i == NT - 1),
                    )
                o_sb = xb.tile([1, D], f32, name=f"o_sb{b}", tag="o_sb")
                nc.vector.tensor_copy(o_sb[:], o_ps[:])
                nc.sync.dma_start(out=o_d[b : b + 1, :], in_=o_sb[:])

    nc.compile()
    return nc


def _get_nc():
    import os

    stage = int(os.environ.get("K_STAGE", "4"))
    batches = int(os.environ.get("K_BATCHES", str(BPC)))
    key = ("nc", stage, batches)
    if key not in _CACHE:
        _CACHE[key] = _build_nc(stage=stage, batches=batches)
    return _CACHE[key]


def _in_maps(inputs):
    x = np.ascontiguousarray(np.asarray(inputs["input"], dtype=np.float32))
    m = np.ascontiguousarray(np.asarray(inputs["mask"], dtype=np.float32))
    assert x.shape == (B, L, D) and m.shape == (B, L)
    return [
        {
            "input": np.ascontiguousarray(x[c * BPC : (c + 1) * BPC]),
            "mask": np.ascontiguousarray(m[c * BPC : (c + 1) * BPC]),
        }
        for c in range(NCORES)
    ]


def _enable_tracing():
    """Shim antenv.axon_hooks (absent in this container) so
    run_bass_kernel_spmd(trace=True) can capture NTFF profiles through
    the axon .so, and neutralize the S3 artifact upload."""
    if _CACHE.get("trace_shim"):
        return
    import types

    import antenv

    if not hasattr(antenv, "axon_hooks"):
        mod = types.ModuleType("antenv.axon_hooks")
        mod._hook = None

        def set_axon_ntff_profile_hook(h):
            mod._hook = h

        def get_axon_ntff_profile_hook():
            return mod._hook

        mod.set_axon_ntff_profile_hook = set_axon_ntff_profile_hook
        mod.get_axon_ntff_profile_hook = get_axon_ntff_profile_hook
        sys.modules["antenv.axon_hooks"] = mod
        antenv.axon_hooks = mod

    from antenv.axon_hooks import get_axon_ntff_profile_hook, set_axon_ntff_profile_hook

    if get_axon_ntff_profile_hook() is None:
        if "/root/.axon_site" not in sys.path:
            sys.path.insert(0, "/root/.axon_site")
        from trn_agent_boot.trn_boot import _ntff_profile_via_ctypes

        set_axon_ntff_profile_hook(
            _ntff_profile_via_ctypes("/opt/axon/libaxon_pjrt.so")
        )

    import concourse.bass_utils as bu

    bu.upload_artifacts = lambda tmpdir: f"local://{tmpdir}"
    _CACHE["trace_shim"] = True


def _run(inputs, trace=False, **kw):
    from concourse.bass_utils import run_bass_kernel_spmd

    if trace:
        _enable_tracing()
    nc = _get_nc()
    res = run_bass_kernel_spmd(
        nc, _in_maps(inputs), core_ids=list(range(NCORES)), trace=trace, **kw
    )
    outs = np.stack([res.results[c]["out"] for c in range(NCORES)])  # [8, BPC, D]
    out_full = outs.reshape(B, 1, D).astype(np.float32)
    attn_mean = np.full((B, L), 1.0 / L, dtype=np.float32)
    return (out_full, attn_mean), res


def kernel(**inputs):
    (out_full, attn_mean), _ = _run(inputs, trace=False)
    return (out_full, attn_mean)


# revision 28
# speedup vs baseline: 1.0593x; 1.0163x over previous
"""Trainium2 Bass kernel for AttnSum3d pooling.

Math (per batch):
    xm = input * mask[:, None]                      # [L, D]
    S  = xm @ xm.T                                  # [L, L], symmetric
    w  = softmax(S, axis=0)  (columns sum to 1 over rows)
    out[d]       = (1/L) * sum_m sum_l w[l, m] xm[l, d]
    attn_mean[m] = (1/L) * sum_l w[l, m] = 1/L      (constant!)

Per m-block of 128 columns (stored row-wise thanks to symmetry):
    c[m]      >= max_l S[l, m]  via  sqrt(nsq[m]*max nsq) (Cauchy-Schwarz),
                 computed with a DVE exponent-halving sqrt (x1.06 margin)
    P_j[m, l] = exp(S_j[m, l] - c[m])     (ACT, bias=-c, accum_out=colsum)
    v[m]      = 1 / (L * colsum[m])
    r[l]     += sum_m v[m] * P_j[m, l]    (TensorE, lhsT=v, bf16)
    out[d]    = sum_l r[l] * xm[l, d]     (DVE accumulation + one
                                           partition-sum matmul)

Data-parallel over batch: 16 batches -> 8 cores x 2 batches. Batch 1's
prep and batch 0's tail are interleaved into the main loops to keep the
ACT engine (the bottleneck) fed continuously.
"""

import sys

for _p in ("/opt/trn_rl_repo",):
    if _p not in sys.path:
        sys.path.insert(0, _p)

import numpy as np

B, L, D = 16, 2048, 128
NCORES = 8
BPC = B // NCORES          # batches per core
NT = L // 128              # 16 tiles of 128 along L
TPB = L // 128             # L-rows per partition in the (p t) layout

_CACHE = {}


def _build_nc(batches=BPC):
    import concourse.bacc as bacc
    import concourse.tile as tile
    from concourse import mybir
    from concourse.masks import make_identity

    f32 = mybir.dt.float32
    bf16 = mybir.dt.bfloat16
    AF = mybir.ActivationFunctionType
    ALU = mybir.AluOpType
    AX = mybir.AxisListType

    nc = bacc.Bacc("TRN2", target_bir_lowering=False, debug=False)

    x_d = nc.dram_tensor("input", [BPC, L, D], f32, kind="ExternalInput").ap()
    m_d = nc.dram_tensor("mask", [BPC, L], f32, kind="ExternalInput").ap()
    o_d = nc.dram_tensor("out", [BPC, D], f32, kind="ExternalOutput").ap()

    with tile.TileContext(nc) as tc:
        with (
            tc.tile_pool(name="consts", bufs=1) as consts,
            tc.tile_pool(name="xb", bufs=2) as xb,
            tc.tile_pool(name="pb", bufs=6) as pb,
            tc.tile_pool(name="small", bufs=4) as small,
            tc.tile_pool(name="acc", bufs=2) as accp,
            tc.tile_pool(name="psS", bufs=2, space="PSUM") as psS,
            tc.tile_pool(name="psR", bufs=1, space="PSUM") as psR,
        ):
            identity = consts.tile([128, 128], f32)
            make_identity(nc, identity)
            ones_row = consts.tile([1, 128], f32)
            nc.vector.memset(ones_row, 1.0)
            ones_col = consts.tile([128, 1], f32)
            nc.vector.memset(ones_col, 1.0)

            ctxs = [{} for _ in range(batches)]
            deferred = []

            def drain(k):
                for _ in range(k):
                    if deferred:
                        deferred.pop(0)()

            def emit_loads(b):
                c = ctxs[b]
                c["xpa"] = xb.tile([128, 8, D], f32, name=f"xpa{b}", tag="xpa")
                c["xpb"] = xb.tile([128, 8, D], f32, name=f"xpb{b}", tag="xpb")
                xsrc = x_d[b].rearrange("(p t) d -> p t d", p=128)
                nc.sync.dma_start(out=c["xpa"][:], in_=xsrc[:, 0:8, :])
                nc.sync.dma_start(out=c["xpb"][:], in_=xsrc[:, 8:16, :])
                c["mask"] = xb.tile([128, TPB], f32, name=f"mask{b}", tag="mask")
                nc.sync.dma_start(
                    out=c["mask"][:], in_=m_d[b].rearrange("(p t) -> p t", p=128)
                )

            def xp_t(c, t):
                return c["xpa"][:, t, :] if t < 8 else c["xpb"][:, t - 8, :]

            def prep_ops(b):
                """Closures for mask-mult, nsq, negc, xmT transposes."""
                c = ctxs[b]
                c["xm"] = xb.tile([128, NT, D], f32, name=f"xm{b}", tag="xm")
                c["nsq"] = xb.tile([128, NT], f32, name=f"nsq{b}", tag="nsq")
                c["xmT"] = xb.tile([128, L], bf16, name=f"xmT{b}", tag="xmT")
                ops = []

                for t in range(NT):
                    def _mask(t=t):
                        nc.vector.tensor_scalar_mul(
                            c["xm"][:, t, :], xp_t(c, t), c["mask"][:, t : t + 1]
                        )
                    ops.append(_mask)

                # nsq: batch 0 on the idle ACT (also pre-warms the exp
                # table); later batches on DVE (ACT is busy by then)
                for t in range(NT):
                    if b == 0:
                        def _nsq(t=t):
                            sqj = small.tile(
                                [128, D], f32, name=f"sqj{b}_{t}", tag="sqj"
                            )
                            nc.scalar.activation(
                                sqj[:],
                                xp_t(c, t),
                                AF.Square,
                                scale=c["mask"][:, t : t + 1],
                                accum_out=c["nsq"][:, t : t + 1],
                            )
                        ops.append(_nsq)
                    else:
                        def _nsq(t=t):
                            sqj = small.tile(
                                [128, D], f32, name=f"sqj{b}_{t}", tag="sqj"
                            )
                            nc.vector.tensor_mul(
                                sqj[:], c["xm"][:, t, :], c["xm"][:, t, :]
                            )
                            nc.vector.reduce_sum(
                                c["nsq"][:, t : t + 1], sqj[:], AX.X
                            )
                        ops.append(_nsq)

                def _n2max_a():
                    c["nmaxp"] = xb.tile([128, 1], f32, name=f"nmx{b}", tag="nmx")
                    nc.vector.reduce_max(c["nmaxp"][:], c["nsq"][:], AX.X)
                    tps = psS.tile([1, 128], f32, name=f"tpn{b}", tag="ps")
                    nc.tensor.transpose(tps[:], c["nmaxp"][:], identity[:])
                    c["nmr"] = xb.tile([1, 128], f32, name=f"nmr{b}", tag="nmr")
                    nc.vector.tensor_copy(c["nmr"][:], tps[:])
                ops.append(_n2max_a)

                def _n2max_b():
                    n2max = xb.tile([1, 1], f32, name=f"n2m{b}", tag="n2m")
                    nc.vector.reduce_max(n2max[:], c["nmr"][:], AX.X)
                    bps = psS.tile([128, 1], f32, name=f"bps{b}", tag="ps")
                    nc.tensor.matmul(
                        bps[:], ones_row[:], n2max[:], start=True, stop=True
                    )
                    c["n2b"] = xb.tile([128, 1], f32, name=f"n2b{b}", tag="n2b")
                    nc.vector.tensor_copy(c["n2b"][:], bps[:])
                ops.append(_n2max_b)

                def _negc():
                    zt = xb.tile([128, NT], f32, name=f"zt{b}", tag="zt")
                    nc.vector.tensor_scalar_mul(zt[:], c["nsq"][:], c["n2b"][:, 0:1])
                    zi = zt[:].bitcast(mybir.dt.int32)
                    nc.vector.tensor_scalar(zi, zi, 1, None, op0=ALU.arith_shift_right)
                    nc.vector.tensor_scalar(zi, zi, 0x1FC00000, None, op0=ALU.add)
                    c["negc"] = xb.tile([128, NT], f32, name=f"negc{b}", tag="negc")
                    nc.vector.tensor_scalar_mul(c["negc"][:], zt[:], -1.06)
                ops.append(_negc)

                for t in range(NT):
                    def _tr(t=t):
                        tp = psS.tile([128, 128], f32, name=f"tp{b}_{t}", tag="ps")
                        nc.tensor.transpose(tp[:], c["xm"][:, t, :], identity[:])
                        nc.vector.tensor_copy(
                            c["xmT"][:, t * 128 : (t + 1) * 128], tp[:]
                        )
                    ops.append(_tr)
                return ops

            def emit_main(b):
                c = ctxs[b]
                c["r_ps"] = psR.tile([1, L], f32, name=f"r_ps{b}", tag="r")
                xmT = c["xmT"]
                for jb in range(NT):
                    lhsT = xmT[:, jb * 128 : (jb + 1) * 128]
                    csum = small.tile([128, 2], f32, name=f"cs{b}_{jb}", tag="cs")
                    Ph = []
                    for h in range(2):
                        S_ps = psS.tile(
                            [128, 1024], f32, name=f"S{b}_{jb}_{h}", tag="ps"
                        )
                        for k in range(2):
                            nc.tensor.matmul(
                                S_ps[:, k * 512 : (k + 1) * 512],
                                lhsT,
                                xmT[
                                    :, h * 1024 + k * 512 : h * 1024 + (k + 1) * 512
                                ],
                                start=True,
                                stop=True,
                            )
                        P = pb.tile([128, 1024], bf16, name=f"P{b}_{jb}_{h}", tag="P")
                        nc.scalar.activation(
                            P[:],
                            S_ps[:],
                            AF.Exp,
                            bias=c["negc"][:, jb : jb + 1],
                            scale=1.0,
                            accum_out=csum[:, h : h + 1],
                        )
                        Ph.append(P)

                    cst = small.tile([128, 1], f32, name=f"cst{b}_{jb}", tag="cst")
                    nc.vector.tensor_add(cst[:], csum[:, 0:1], csum[:, 1:2])
                    vj = small.tile([128, 1], f32, name=f"vj{b}_{jb}", tag="vj")
                    nc.vector.reciprocal(vj[:], cst[:])
                    vjb = small.tile([128, 1], bf16, name=f"vjb{b}_{jb}", tag="vjb")
                    nc.vector.tensor_scalar_mul(vjb[:], vj[:], 1.0 / L)

                    for h in range(2):
                        for k in range(2):
                            nc.tensor.matmul(
                                c["r_ps"][
                                    0:1, h * 1024 + k * 512 : h * 1024 + (k + 1) * 512
                                ],
                                vjb[:],
                                Ph[h][:, k * 512 : (k + 1) * 512],
                                start=(jb == 0),
                                stop=(jb == NT - 1),
                            )
                    drain(5)

                # drain r to SBUF per chunk (releases the psR slot)
                c["r_sb"] = xb.tile([1, L], f32, name=f"r_sb{b}", tag="r_sb")
                for q in range(4):
                    nc.vector.tensor_copy(
                        c["r_sb"][0:1, q * 512 : (q + 1) * 512],
                        c["r_ps"][0:1, q * 512 : (q + 1) * 512],
                    )

            def tail_ops(b):
                """rT transposes + DVE accumulation of out = r @ xm."""
                c = ctxs[b]
                c["rT"] = xb.tile([128, NT], f32, name=f"rT{b}", tag="rT")
                ops = []
                for i in range(NT):
                    def _rt(i=i):
                        tpr = psS.tile([128, 1], f32, name=f"tpr{b}_{i}", tag="ps")
                        nc.tensor.transpose(
                            tpr[:],
                            c["r_sb"][0:1, i * 128 : (i + 1) * 128],
                            identity[0:1, 0:1],
                        )
                        nc.vector.tensor_copy(c["rT"][:, i : i + 1], tpr[:])
                    ops.append(_rt)

                def _acc0():
                    a = accp.tile([128, D], f32, name=f"acc{b}_0", tag=f"acc{b}")
                    nc.vector.tensor_scalar_mul(
                        a[:], c["xm"][:, 0, :], c["rT"][:, 0:1]
                    )
                    c["acc"] = a
                ops.append(_acc0)
                for i in range(1, NT):
                    def _acci(i=i):
                        a = accp.tile([128, D], f32, name=f"acc{b}_{i}", tag=f"acc{b}")
                        nc.vector.scalar_tensor_tensor(
                            out=a[:],
                            in0=c["xm"][:, i, :],
                            scalar=c["rT"][:, i : i + 1],
                            in1=c["acc"][:],
                            op0=ALU.mult,
                            op1=ALU.add,
                        )
                        c["acc"] = a
                    ops.append(_acci)

                def _fin():
                    o_ps = psS.tile([1, D], f32, name=f"o_ps{b}", tag="ps")
                    nc.tensor.matmul(
                        o_ps[:], ones_col[:], c["acc"][:], start=True, stop=True
                    )
                    o_sb = xb.tile([1, D], f32, name=f"o_sb{b}", tag="o_sb")
                    nc.vector.tensor_copy(o_sb[:], o_ps[:])
                    nc.sync.dma_start(out=o_d[b : b + 1, :], in_=o_sb[:])
                ops.append(_fin)
                return ops

            # ---------------- emission schedule ----------------
            for b in range(batches):
                emit_loads(b)
            for op in prep_ops(0):
                op()
            for b in range(batches):
                if b + 1 < batches:
                    deferred.extend(prep_ops(b + 1))
                emit_main(b)  # drains deferred (prep of b+1 / tail of b-1)
                deferred.extend(tail_ops(b))
            while deferred:
                deferred.pop(0)()

    nc.compile()
    return nc


def _get_nc():
    import os

    batches = int(os.environ.get("K_BATCHES", str(BPC)))
    key = ("nc", batches)
    if key not in _CACHE:
        _CACHE[key] = _build_nc(batches=batches)
    return _CACHE[key]


def _enable_tracing():
    """Shim antenv.axon_hooks (absent in this container) so
    run_bass_kernel_spmd(trace=True) can capture NTFF profiles through
    the axon .so, and neutralize the S3 artifact upload."""
    if _CACHE.get("trace_shim"):
        return
    import types

    import antenv

    if not hasattr(antenv, "axon_hooks"):
        mod = types.ModuleType("antenv.axon_hooks")
        mod._hook = None

        def set_axon_ntff_profile_hook(h):
            mod._hook = h

        def get_axon_ntff_profile_hook():
            return mod._hook

        mod.set_axon_ntff_profile_hook = set_axon_ntff_profile_hook
        mod.get_axon_ntff_profile_hook = get_axon_ntff_profile_hook
        sys.modules["antenv.axon_hooks"] = mod
        antenv.axon_hooks = mod

    from antenv.axon_hooks import get_axon_ntff_profile_hook, set_axon_ntff_profile_hook

    if get_axon_ntff_profile_hook() is None:
        if "/root/.axon_site" not in sys.path:
            sys.path.insert(0, "/root/.axon_site")
        from trn_agent_boot.trn_boot import _ntff_profile_via_ctypes

        set_axon_ntff_profile_hook(
            _ntff_profile_via_ctypes("/opt/axon/libaxon_pjrt.so")
        )

    import concourse.bass_utils as bu

    bu.upload_artifacts = lambda tmpdir: f"local://{tmpdir}"
    _CACHE["trace_shim"] = True


def _in_maps(inputs):
    x = np.ascontiguousarray(np.asarray(inputs["input"], dtype=np.float32))
    m = np.ascontiguousarray(np.asarray(inputs["mask"], dtype=np.float32))
    assert x.shape == (B, L, D) and m.shape == (B, L)
    return [
        {
            "input": np.ascontiguousarray(x[c * BPC : (c + 1) * BPC]),
            "mask": np.ascontiguousarray(m[c * BPC : (c + 1) * BPC]),
        }
        for c in range(NCORES)
    ]


def _run(inputs, trace=False, **kw):
    from concourse.bass_utils import run_bass_kernel_spmd

    if trace:
        _enable_tracing()
    nc = _get_nc()
    res = run_bass_kernel_spmd(
        nc, _in_maps(inputs), core_ids=list(range(NCORES)), trace=trace, **kw
    )
    outs = np.stack([res.results[c]["out"] for c in range(NCORES)])  # [8, BPC, D]
    out_full = outs.reshape(B, 1, D).astype(np.float32)
    attn_mean = np.full((B, L), 1.0 / L, dtype=np.float32)
    return (out_full, attn_mean), res


def kernel(**inputs):
    (out_full, attn_mean), _ = _run(inputs, trace=False)
    return (out_full, attn_mean)


# revision 29
# speedup vs baseline: 1.0839x; 1.0232x over previous
"""Trainium2 Bass kernel for AttnSum3d pooling.

Math (per batch):
    xm = input * mask[:, None]                      # [L, D]
    S  = xm @ xm.T                                  # [L, L], symmetric
    w  = softmax(S, axis=0)  (columns sum to 1 over rows)
    out[d]       = (1/L) * sum_m sum_l w[l, m] xm[l, d]
    attn_mean[m] = (1/L) * sum_l w[l, m] = 1/L      (constant!)

Per m-block of 128 columns (stored row-wise thanks to symmetry):
    c[m]      >= max_l S[l, m]  via  sqrt(nsq[m]*max nsq) (Cauchy-Schwarz),
                 computed with a DVE exponent-halving sqrt (x1.06 margin)
    P_j[m, l] = exp(S_j[m, l] - c[m])     (ACT, bias=-c, accum_out=colsum)
    v[m]      = 1 / (L * colsum[m])
    r[l]     += sum_m v[m] * P_j[m, l]    (TensorE, lhsT=v, bf16)
    out[d]    = sum_l r[l] * xm[l, d]     (DVE accumulation + one
                                           partition-sum matmul)

Data-parallel over batch: 16 batches -> 8 cores x 2 batches. Batch 1's
prep and batch 0's tail are interleaved into the main loops to keep the
ACT engine (the bottleneck) fed continuously.
"""

import sys

for _p in ("/opt/trn_rl_repo",):
    if _p not in sys.path:
        sys.path.insert(0, _p)

import numpy as np

B, L, D = 16, 2048, 128
NCORES = 8
BPC = B // NCORES          # batches per core
NT = L // 128              # 16 tiles of 128 along L
TPB = L // 128             # L-rows per partition in the (p t) layout

_CACHE = {}


def _build_nc(batches=BPC):
    import concourse.bacc as bacc
    import concourse.tile as tile
    from concourse import mybir
    from concourse.masks import make_identity

    f32 = mybir.dt.float32
    bf16 = mybir.dt.bfloat16
    AF = mybir.ActivationFunctionType
    ALU = mybir.AluOpType
    AX = mybir.AxisListType

    nc = bacc.Bacc("TRN2", target_bir_lowering=False, debug=False)

    x_d = nc.dram_tensor("input", [BPC, L, D], f32, kind="ExternalInput").ap()
    m_d = nc.dram_tensor("mask", [BPC, L], f32, kind="ExternalInput").ap()
    o_d = nc.dram_tensor("out", [BPC, D], f32, kind="ExternalOutput").ap()

    with tile.TileContext(nc) as tc:
        with (
            tc.tile_pool(name="consts", bufs=1) as consts,
            tc.tile_pool(name="xb", bufs=2) as xb,
            tc.tile_pool(name="pb", bufs=6) as pb,
            tc.tile_pool(name="small", bufs=4) as small,
            tc.tile_pool(name="acc", bufs=2) as accp,
            tc.tile_pool(name="psS", bufs=2, space="PSUM") as psS,
            tc.tile_pool(name="psR", bufs=1, space="PSUM") as psR,
        ):
            identity = consts.tile([128, 128], f32)
            make_identity(nc, identity)
            ones_row = consts.tile([1, 128], f32)
            nc.vector.memset(ones_row, 1.0)
            ones_col = consts.tile([128, 1], f32)
            nc.vector.memset(ones_col, 1.0)

            ctxs = [{} for _ in range(batches)]
            deferred = []

            def drain(k):
                for _ in range(k):
                    if deferred:
                        deferred.pop(0)()

            def emit_loads(b):
                c = ctxs[b]
                c["xpa"] = xb.tile([128, 8, D], f32, name=f"xpa{b}", tag="xpa")
                c["xpb"] = xb.tile([128, 8, D], f32, name=f"xpb{b}", tag="xpb")
                xsrc = x_d[b].rearrange("(p t) d -> p t d", p=128)
                nc.sync.dma_start(out=c["xpa"][:], in_=xsrc[:, 0:8, :])
                nc.sync.dma_start(out=c["xpb"][:], in_=xsrc[:, 8:16, :])
                c["mask"] = xb.tile([128, TPB], f32, name=f"mask{b}", tag="mask")
                nc.sync.dma_start(
                    out=c["mask"][:], in_=m_d[b].rearrange("(p t) -> p t", p=128)
                )

            def xp_t(c, t):
                return c["xpa"][:, t, :] if t < 8 else c["xpb"][:, t - 8, :]

            def prep_ops(b):
                """Closures for mask-mult, nsq, negc, xmT transposes."""
                c = ctxs[b]
                c["xm"] = xb.tile([128, NT, D], f32, name=f"xm{b}", tag="xm")
                c["nsq"] = xb.tile([128, NT], f32, name=f"nsq{b}", tag="nsq")
                c["xmT"] = xb.tile([128, L], bf16, name=f"xmT{b}", tag="xmT")
                ops = []

                for t in range(NT):
                    def _mask(t=t):
                        nc.vector.tensor_scalar_mul(
                            c["xm"][:, t, :], xp_t(c, t), c["mask"][:, t : t + 1]
                        )
                    ops.append(_mask)

                # nsq: batch 0 on the idle ACT (also pre-warms the exp
                # table); later batches on DVE (ACT is busy by then)
                for t in range(NT):
                    if b == 0:
                        def _nsq(t=t):
                            sqj = small.tile(
                                [128, D], f32, name=f"sqj{b}_{t}", tag="sqj"
                            )
                            nc.scalar.activation(
                                sqj[:],
                                xp_t(c, t),
                                AF.Square,
                                scale=c["mask"][:, t : t + 1],
                                accum_out=c["nsq"][:, t : t + 1],
                            )
                        ops.append(_nsq)
                    else:
                        def _nsq(t=t):
                            sqj = small.tile(
                                [128, D], f32, name=f"sqj{b}_{t}", tag="sqj"
                            )
                            nc.vector.tensor_mul(
                                sqj[:], c["xm"][:, t, :], c["xm"][:, t, :]
                            )
                            nc.vector.reduce_sum(
                                c["nsq"][:, t : t + 1], sqj[:], AX.X
                            )
                        ops.append(_nsq)

                def _n2max_a():
                    c["nmaxp"] = xb.tile([128, 1], f32, name=f"nmx{b}", tag="nmx")
                    nc.vector.reduce_max(c["nmaxp"][:], c["nsq"][:], AX.X)
                    tps = psS.tile([1, 128], f32, name=f"tpn{b}", tag="ps")
                    nc.tensor.transpose(tps[:], c["nmaxp"][:], identity[:])
                    c["nmr"] = xb.tile([1, 128], f32, name=f"nmr{b}", tag="nmr")
                    nc.vector.tensor_copy(c["nmr"][:], tps[:])
                ops.append(_n2max_a)

                def _n2max_b():
                    n2max = xb.tile([1, 1], f32, name=f"n2m{b}", tag="n2m")
                    nc.vector.reduce_max(n2max[:], c["nmr"][:], AX.X)
                    bps = psS.tile([128, 1], f32, name=f"bps{b}", tag="ps")
                    nc.tensor.matmul(
                        bps[:], ones_row[:], n2max[:], start=True, stop=True
                    )
                    c["n2b"] = xb.tile([128, 1], f32, name=f"n2b{b}", tag="n2b")
                    nc.vector.tensor_copy(c["n2b"][:], bps[:])
                ops.append(_n2max_b)

                def _negc():
                    zt = xb.tile([128, NT], f32, name=f"zt{b}", tag="zt")
                    nc.vector.tensor_scalar_mul(zt[:], c["nsq"][:], c["n2b"][:, 0:1])
                    zi = zt[:].bitcast(mybir.dt.int32)
                    nc.vector.tensor_scalar(zi, zi, 1, None, op0=ALU.arith_shift_right)
                    nc.vector.tensor_scalar(zi, zi, 0x1FC00000, None, op0=ALU.add)
                    c["negc"] = xb.tile([128, NT], f32, name=f"negc{b}", tag="negc")
                    nc.vector.tensor_scalar_mul(c["negc"][:], zt[:], -1.06)
                ops.append(_negc)

                for t in range(NT):
                    def _tr(t=t):
                        tp = psS.tile([128, 128], f32, name=f"tp{b}_{t}", tag="ps")
                        nc.tensor.transpose(tp[:], c["xm"][:, t, :], identity[:])
                        nc.vector.tensor_copy(
                            c["xmT"][:, t * 128 : (t + 1) * 128], tp[:]
                        )
                    ops.append(_tr)
                return ops

            def emit_main(b):
                c = ctxs[b]
                c["r_ps"] = psR.tile([1, L], f32, name=f"r_ps{b}", tag="r")
                xmT = c["xmT"]
                pend = {}  # jb -> (vjb, Ph): vP runs one block behind so the
                # PE never waits on the exp/v-chain of the current block

                def emit_vP(jb):
                    vjb, Ph = pend.pop(jb)
                    for h in range(2):
                        for k in range(2):
                            nc.tensor.matmul(
                                c["r_ps"][
                                    0:1, h * 1024 + k * 512 : h * 1024 + (k + 1) * 512
                                ],
                                vjb[:],
                                Ph[h][:, k * 512 : (k + 1) * 512],
                                start=(jb == 0),
                                stop=(jb == NT - 1),
                            )

                for jb in range(NT):
                    lhsT = xmT[:, jb * 128 : (jb + 1) * 128]
                    csum = small.tile([128, 2], f32, name=f"cs{b}_{jb}", tag="cs")
                    Ph = []
                    for h in range(2):
                        S_ps = psS.tile(
                            [128, 1024], f32, name=f"S{b}_{jb}_{h}", tag="ps"
                        )
                        for k in range(2):
                            nc.tensor.matmul(
                                S_ps[:, k * 512 : (k + 1) * 512],
                                lhsT,
                                xmT[
                                    :, h * 1024 + k * 512 : h * 1024 + (k + 1) * 512
                                ],
                                start=True,
                                stop=True,
                            )
                        P = pb.tile([128, 1024], bf16, name=f"P{b}_{jb}_{h}", tag="P")
                        nc.scalar.activation(
                            P[:],
                            S_ps[:],
                            AF.Exp,
                            bias=c["negc"][:, jb : jb + 1],
                            scale=1.0,
                            accum_out=csum[:, h : h + 1],
                        )
                        Ph.append(P)

                    cst = small.tile([128, 1], f32, name=f"cst{b}_{jb}", tag="cst")
                    nc.vector.tensor_add(cst[:], csum[:, 0:1], csum[:, 1:2])
                    vj = small.tile([128, 1], f32, name=f"vj{b}_{jb}", tag="vj")
                    nc.vector.reciprocal(vj[:], cst[:])
                    vjb = small.tile([128, 1], bf16, name=f"vjb{b}_{jb}", tag="vjb")
                    nc.vector.tensor_scalar_mul(vjb[:], vj[:], 1.0 / L)
                    pend[jb] = (vjb, Ph)
                    if jb > 0:
                        emit_vP(jb - 1)
                    drain(5)
                emit_vP(NT - 1)

                # drain r to SBUF per chunk (releases the psR slot)
                c["r_sb"] = xb.tile([1, L], f32, name=f"r_sb{b}", tag="r_sb")
                for q in range(4):
                    nc.vector.tensor_copy(
                        c["r_sb"][0:1, q * 512 : (q + 1) * 512],
                        c["r_ps"][0:1, q * 512 : (q + 1) * 512],
                    )

            def tail_ops(b):
                """rT transposes + DVE accumulation of out = r @ xm."""
                c = ctxs[b]
                c["rT"] = xb.tile([128, NT], f32, name=f"rT{b}", tag="rT")
                ops = []
                for i in range(NT):
                    def _rt(i=i):
                        tpr = psS.tile([128, 1], f32, name=f"tpr{b}_{i}", tag="ps")
                        nc.tensor.transpose(
                            tpr[:],
                            c["r_sb"][0:1, i * 128 : (i + 1) * 128],
                            identity[0:1, 0:1],
                        )
                        nc.vector.tensor_copy(c["rT"][:, i : i + 1], tpr[:])
                    ops.append(_rt)

                def _acc0():
                    a = accp.tile([128, D], f32, name=f"acc{b}_0", tag=f"acc{b}")
                    nc.vector.tensor_scalar_mul(
                        a[:], c["xm"][:, 0, :], c["rT"][:, 0:1]
                    )
                    c["acc"] = a
                ops.append(_acc0)
                for i in range(1, NT):
                    def _acci(i=i):
                        a = accp.tile([128, D], f32, name=f"acc{b}_{i}", tag=f"acc{b}")
                        nc.vector.scalar_tensor_tensor(
                            out=a[:],
                            in0=c["xm"][:, i, :],
                            scalar=c["rT"][:, i : i + 1],
                            in1=c["acc"][:],
                            op0=ALU.mult,
                            op1=ALU.add,
                        )
                        c["acc"] = a
                    ops.append(_acci)

                def _fin():
                    o_ps = psS.tile([1, D], f32, name=f"o_ps{b}", tag="ps")
                    nc.tensor.matmul(
                        o_ps[:], ones_col[:], c["acc"][:], start=True, stop=True
                    )
                    o_sb = xb.tile([1, D], f32, name=f"o_sb{b}", tag="o_sb")
                    nc.vector.tensor_copy(o_sb[:], o_ps[:])
                    nc.sync.dma_start(out=o_d[b : b + 1, :], in_=o_sb[:])
                ops.append(_fin)
                return ops

            # ---------------- emission schedule ----------------
            for b in range(batches):
                emit_loads(b)
            for op in prep_ops(0):
                op()
            for b in range(batches):
                if b + 1 < batches:
                    deferred.extend(prep_ops(b + 1))
                emit_main(b)  # drains deferred (prep of b+1 / tail of b-1)
                deferred.extend(tail_ops(b))
            while deferred:
                deferred.pop(0)()

    nc.compile()
    return nc


def _get_nc():
    import os

    batches = int(os.environ.get("K_BATCHES", str(BPC)))
    key = ("nc", batches)
    if key not in _CACHE:
        _CACHE[key] = _build_nc(batches=batches)
    return _CACHE[key]


def _enable_tracing():
    """Shim antenv.axon_hooks (absent in this container) so
    run_bass_kernel_spmd(trace=True) can capture NTFF profiles through
    the axon .so, and neutralize the S3 artifact upload."""
    if _CACHE.get("trace_shim"):
        return
    import types

    import antenv

    if not hasattr(antenv, "axon_hooks"):
        mod = types.ModuleType("antenv.axon_hooks")
        mod._hook = None

        def set_axon_ntff_profile_hook(h):
            mod._hook = h

        def get_axon_ntff_profile_hook():
            return mod._hook

        mod.set_axon_ntff_profile_hook = set_axon_ntff_profile_hook
        mod.get_axon_ntff_profile_hook = get_axon_ntff_profile_hook
        sys.modules["antenv.axon_hooks"] = mod
        antenv.axon_hooks = mod

    from antenv.axon_hooks import get_axon_ntff_profile_hook, set_axon_ntff_profile_hook

    if get_axon_ntff_profile_hook() is None:
        if "/root/.axon_site" not in sys.path:
            sys.path.insert(0, "/root/.axon_site")
        from trn_agent_boot.trn_boot import _ntff_profile_via_ctypes

        set_axon_ntff_profile_hook(
            _ntff_profile_via_ctypes("/opt/axon/libaxon_pjrt.so")
        )

    import concourse.bass_utils as bu

    bu.upload_artifacts = lambda tmpdir: f"local://{tmpdir}"
    _CACHE["trace_shim"] = True


def _in_maps(inputs):
    x = np.ascontiguousarray(np.asarray(inputs["input"], dtype=np.float32))
    m = np.ascontiguousarray(np.asarray(inputs["mask"], dtype=np.float32))
    assert x.shape == (B, L, D) and m.shape == (B, L)
    return [
        {
            "input": np.ascontiguousarray(x[c * BPC : (c + 1) * BPC]),
            "mask": np.ascontiguousarray(m[c * BPC : (c + 1) * BPC]),
        }
        for c in range(NCORES)
    ]


def _run(inputs, trace=False, **kw):
    from concourse.bass_utils import run_bass_kernel_spmd

    if trace:
        _enable_tracing()
    nc = _get_nc()
    res = run_bass_kernel_spmd(
        nc, _in_maps(inputs), core_ids=list(range(NCORES)), trace=trace, **kw
    )
    outs = np.stack([res.results[c]["out"] for c in range(NCORES)])  # [8, BPC, D]
    out_full = outs.reshape(B, 1, D).astype(np.float32)
    attn_mean = np.full((B, L), 1.0 / L, dtype=np.float32)
    return (out_full, attn_mean), res


def kernel(**inputs):
    (out_full, attn_mean), _ = _run(inputs, trace=False)
    return (out_full, attn_mean)


# revision 38
# speedup vs baseline: 1.1671x; 1.0768x over previous
"""Trainium2 Bass kernel for AttnSum3d pooling.

Math (per batch):
    xm = input * mask[:, None]                      # [L, D]
    S  = xm @ xm.T                                  # [L, L], symmetric
    w  = softmax(S, axis=0)  (columns sum to 1 over rows)
    out[d]       = (1/L) * sum_m sum_l w[l, m] xm[l, d]
    attn_mean[m] = (1/L) * sum_l w[l, m] = 1/L      (constant!)

Per m-block of 128 columns (stored row-wise thanks to symmetry):
    c[m]      >= max_l S[l, m]  via  sqrt(nsq[m]*max nsq) (Cauchy-Schwarz),
                 computed with a DVE exponent-halving sqrt (x1.06 margin)
    P_j[m, l] = exp(S_j[m, l] - c[m])     (ACT, bias=-c, accum_out=colsum)
    v[m]      = 1 / (L * colsum[m])
    r[l]     += sum_m v[m] * P_j[m, l]    (TensorE, lhsT=v, bf16)
    out[d]    = sum_l r[l] * xm[l, d]     (DVE accumulation + one
                                           partition-sum matmul)

Data-parallel over batch: 16 batches -> 8 cores x 2 batches. Batch 1's
prep and batch 0's tail are interleaved into the main loops to keep the
ACT engine (the bottleneck) fed continuously.
"""

import sys

for _p in ("/opt/trn_rl_repo",):
    if _p not in sys.path:
        sys.path.insert(0, _p)

import numpy as np

B, L, D = 16, 2048, 128
NCORES = 8
BPC = B // NCORES          # batches per core
NT = L // 128              # 16 tiles of 128 along L
TPB = L // 128             # L-rows per partition in the (p t) layout

_CACHE = {}


def _build_nc(batches=BPC):
    import concourse.bacc as bacc
    import concourse.tile as tile
    from concourse import mybir
    from concourse.masks import make_identity

    f32 = mybir.dt.float32
    bf16 = mybir.dt.bfloat16
    AF = mybir.ActivationFunctionType
    ALU = mybir.AluOpType
    AX = mybir.AxisListType

    nc = bacc.Bacc("TRN2", target_bir_lowering=False, debug=False)

    x_d = nc.dram_tensor("input", [BPC, L, D], f32, kind="ExternalInput").ap()
    m_d = nc.dram_tensor("mask", [BPC, L], f32, kind="ExternalInput").ap()
    o_d = nc.dram_tensor("out", [BPC, D], f32, kind="ExternalOutput").ap()

    with tile.TileContext(nc) as tc:
        with (
            tc.tile_pool(name="consts", bufs=1) as consts,
            tc.tile_pool(name="xb", bufs=2) as xb,
            tc.tile_pool(name="pb", bufs=6) as pb,
            tc.tile_pool(name="small", bufs=4) as small,
            tc.tile_pool(name="acc", bufs=2) as accp,
            tc.tile_pool(name="psS", bufs=2, space="PSUM") as psS,
            tc.tile_pool(name="psR", bufs=1, space="PSUM") as psR,
        ):
            identity = consts.tile([128, 128], f32)
            make_identity(nc, identity)
            ones_row = consts.tile([1, 128], f32)
            nc.vector.memset(ones_row, 1.0)
            ones_col = consts.tile([128, 1], f32)
            nc.vector.memset(ones_col, 1.0)

            ctxs = [{} for _ in range(batches)]
            # deferred closures, dependency-ordered and hand-interleaved so
            # PE-heavy ops (transposes) spread ~1-2 per block and never
            # crowd out S-matmul production in the static PE schedule
            deferred = []

            def drain(k):
                for _ in range(k):
                    if deferred:
                        deferred.pop(0)()

            def emit_loads(b):
                c = ctxs[b]
                c["xpa"] = xb.tile([128, 8, D], f32, name=f"xpa{b}", tag="xpa")
                c["xpb"] = xb.tile([128, 8, D], f32, name=f"xpb{b}", tag="xpb")
                xsrc = x_d[b].rearrange("(p t) d -> p t d", p=128)
                nc.sync.dma_start(out=c["xpa"][:], in_=xsrc[:, 0:8, :])
                nc.sync.dma_start(out=c["xpb"][:], in_=xsrc[:, 8:16, :])
                c["mask"] = xb.tile([128, TPB], f32, name=f"mask{b}", tag="mask")
                nc.sync.dma_start(
                    out=c["mask"][:], in_=m_d[b].rearrange("(p t) -> p t", p=128)
                )

            def xp_t(c, t):
                return c["xpa"][:, t, :] if t < 8 else c["xpb"][:, t - 8, :]

            def prep_ops(b):
                """Dependency-ordered closures for mask/nsq/negc/xmT with
                PE transposes interleaved between DVE ops."""
                c = ctxs[b]
                c["xm"] = xb.tile([128, NT, D], f32, name=f"xm{b}", tag="xm")
                c["nsq"] = xb.tile([128, NT], f32, name=f"nsq{b}", tag="nsq")
                c["xmT"] = xb.tile([128, L], bf16, name=f"xmT{b}", tag="xmT")
                masks = []
                nsqs = []
                trs = []
                finals = []

                for t in range(NT):
                    def _mask(t=t):
                        nc.vector.tensor_scalar_mul(
                            c["xm"][:, t, :], xp_t(c, t), c["mask"][:, t : t + 1]
                        )
                    masks.append(_mask)

                # nsq: batch 0 on the idle ACT (also pre-warms the exp
                # table); later batches on DVE (ACT is busy by then)
                for t in range(NT):
                    if b == 0:
                        def _nsq(t=t):
                            sqj = small.tile(
                                [128, D], f32, name=f"sqj{b}_{t}", tag="sqj"
                            )
                            nc.scalar.activation(
                                sqj[:],
                                xp_t(c, t),
                                AF.Square,
                                scale=c["mask"][:, t : t + 1],
                                accum_out=c["nsq"][:, t : t + 1],
                            )
                        nsqs.append(_nsq)
                    else:
                        def _nsq(t=t):
                            sqj = small.tile(
                                [128, D], f32, name=f"sqj{b}_{t}", tag="sqj"
                            )
                            nc.vector.tensor_mul(
                                sqj[:], c["xm"][:, t, :], c["xm"][:, t, :]
                            )
                            nc.vector.reduce_sum(
                                c["nsq"][:, t : t + 1], sqj[:], AX.X
                            )
                        nsqs.append(_nsq)

                def _n2max_a():
                    c["nmaxp"] = xb.tile([128, 1], f32, name=f"nmx{b}", tag="nmx")
                    nc.vector.reduce_max(c["nmaxp"][:], c["nsq"][:], AX.X)
                    tps = psS.tile([1, 128], f32, name=f"tpn{b}", tag="ps")
                    nc.tensor.transpose(tps[:], c["nmaxp"][:], identity[:])
                    c["nmr"] = xb.tile([1, 128], f32, name=f"nmr{b}", tag="nmr")
                    nc.vector.tensor_copy(c["nmr"][:], tps[:])
                finals.append(_n2max_a)

                def _n2max_b():
                    n2max = xb.tile([1, 1], f32, name=f"n2m{b}", tag="n2m")
                    nc.vector.reduce_max(n2max[:], c["nmr"][:], AX.X)
                    bps = psS.tile([128, 1], f32, name=f"bps{b}", tag="ps")
                    nc.tensor.matmul(
                        bps[:], ones_row[:], n2max[:], start=True, stop=True
                    )
                    c["n2b"] = xb.tile([128, 1], f32, name=f"n2b{b}", tag="n2b")
                    nc.vector.tensor_copy(c["n2b"][:], bps[:])
                finals.append(_n2max_b)

                def _negc():
                    zt = xb.tile([128, NT], f32, name=f"zt{b}", tag="zt")
                    nc.vector.tensor_scalar_mul(zt[:], c["nsq"][:], c["n2b"][:, 0:1])
                    zi = zt[:].bitcast(mybir.dt.int32)
                    nc.vector.tensor_scalar(zi, zi, 1, None, op0=ALU.arith_shift_right)
                    nc.vector.tensor_scalar(zi, zi, 0x1FC00000, None, op0=ALU.add)
                    c["negc"] = xb.tile([128, NT], f32, name=f"negc{b}", tag="negc")
                    nc.vector.tensor_scalar_mul(c["negc"][:], zt[:], -1.06)
                finals.append(_negc)

                for t in range(NT):
                    def _tr(t=t):
                        tp = psS.tile([128, 128], f32, name=f"tp{b}_{t}", tag="ps")
                        nc.tensor.transpose(tp[:], c["xm"][:, t, :], identity[:])
                        nc.vector.tensor_copy(
                            c["xmT"][:, t * 128 : (t + 1) * 128], tp[:]
                        )
                    trs.append(_tr)
                # interleave: all masks, then alternate nsq/transpose
                ops = list(masks)
                for t in range(NT):
                    ops.append(nsqs[t])
                    ops.append(trs[t])
                ops.extend(finals)
                return ops

            def emit_main(b):
                c = ctxs[b]
                c["r_ps"] = psR.tile([1, L], f32, name=f"r_ps{b}", tag="r")
                xmT = c["xmT"]
                pend = {}  # jb -> (vjb, Ph): vP runs one block behind so the
                # PE never waits on the exp/v-chain of the current block

                def emit_vP(jb):
                    vjb, Ph = pend.pop(jb)
                    for h in range(2):
                        for k in range(2):
                            nc.tensor.matmul(
                                c["r_ps"][
                                    0:1, h * 1024 + k * 512 : h * 1024 + (k + 1) * 512
                                ],
                                vjb[:],
                                Ph[h][:, k * 512 : (k + 1) * 512],
                                start=(jb == 0),
                                stop=(jb == NT - 1),
                            )

                for jb in range(NT):
                    lhsT = xmT[:, jb * 128 : (jb + 1) * 128]
                    csum = small.tile([128, 2], f32, name=f"cs{b}_{jb}", tag="cs")
                    Ph = []
                    for h in range(2):
                        S_ps = psS.tile(
                            [128, 1024], f32, name=f"S{b}_{jb}_{h}", tag="ps"
                        )
                        for k in range(2):
                            nc.tensor.matmul(
                                S_ps[:, k * 512 : (k + 1) * 512],
                                lhsT,
                                xmT[
                                    :, h * 1024 + k * 512 : h * 1024 + (k + 1) * 512
                                ],
                                start=True,
                                stop=True,
                            )
                        P = pb.tile([128, 1024], bf16, name=f"P{b}_{jb}_{h}", tag="P")
                        nc.scalar.activation(
                            P[:],
                            S_ps[:],
                            AF.Exp,
                            bias=c["negc"][:, jb : jb + 1],
                            scale=1.0,
                            accum_out=csum[:, h : h + 1],
                        )
                        Ph.append(P)

                    cst = small.tile([128, 1], f32, name=f"cst{b}_{jb}", tag="cst")
                    nc.vector.tensor_add(cst[:], csum[:, 0:1], csum[:, 1:2])
                    vj = small.tile([128, 1], f32, name=f"vj{b}_{jb}", tag="vj")
                    nc.vector.reciprocal(vj[:], cst[:])
                    vjb = small.tile([128, 1], bf16, name=f"vjb{b}_{jb}", tag="vjb")
                    nc.vector.tensor_scalar_mul(vjb[:], vj[:], 1.0 / L)
                    pend[jb] = (vjb, Ph)
                    if jb > 0:
                        emit_vP(jb - 1)
                    drain(4)
                emit_vP(NT - 1)

                c["r_sb"] = xb.tile([1, L], f32, name=f"r_sb{b}", tag="r_sb")

            def tail_ops(b):
                """r-psum drain + rT transposes + out accumulation, in
                dependency order with PE ops spread out."""
                c = ctxs[b]
                c["rT"] = xb.tile([128, NT], f32, name=f"rT{b}", tag="rT")
                cps = []
                for q in range(4):
                    def _cp(q=q):
                        nc.vector.tensor_copy(
                            c["r_sb"][0:1, q * 512 : (q + 1) * 512],
                            c["r_ps"][0:1, q * 512 : (q + 1) * 512],
                        )
                    cps.append(_cp)
                rts = []
                accs = []
                for i in range(NT):
                    def _rt(i=i):
                        tpr = psS.tile([128, 1], f32, name=f"tpr{b}_{i}", tag="ps")
                        nc.tensor.transpose(
                            tpr[:],
                            c["r_sb"][0:1, i * 128 : (i + 1) * 128],
                            identity[0:1, 0:1],
                        )
                        nc.vector.tensor_copy(c["rT"][:, i : i + 1], tpr[:])
                    rts.append(_rt)

                def _acc0():
                    a = accp.tile([128, D], f32, name=f"acc{b}_0", tag=f"acc{b}")
                    nc.vector.tensor_scalar_mul(
                        a[:], c["xm"][:, 0, :], c["rT"][:, 0:1]
                    )
                    c["acc"] = a
                accs.append(_acc0)
                for i in range(1, NT):
                    def _acci(i=i):
                        a = accp.tile([128, D], f32, name=f"acc{b}_{i}", tag=f"acc{b}")
                        nc.vector.scalar_tensor_tensor(
                            out=a[:],
                            in0=c["xm"][:, i, :],
                            scalar=c["rT"][:, i : i + 1],
                            in1=c["acc"][:],
                            op0=ALU.mult,
                            op1=ALU.add,
                        )
                        c["acc"] = a
                    accs.append(_acci)

                def _fin():
                    o_ps = psS.tile([1, D], f32, name=f"o_ps{b}", tag="ps")
                    nc.tensor.matmul(
                        o_ps[:], ones_col[:], c["acc"][:], start=True, stop=True
                    )
                    o_sb = xb.tile([1, D], f32, name=f"o_sb{b}", tag="o_sb")
                    nc.vector.tensor_copy(o_sb[:], o_ps[:])
                    nc.sync.dma_start(out=o_d[b : b + 1, :], in_=o_sb[:])
                ops = []
                for i in range(NT):
                    if i % 4 == 0:
                        ops.append(cps[i // 4])
                    ops.append(rts[i])
                    ops.append(accs[i])
                ops.append(_fin)
                return ops

            # ---------------- emission schedule ----------------
            for b in range(batches):
                emit_loads(b)
            for op in prep_ops(0):
                op()
            for b in range(batches):
                if b + 1 < batches:
                    deferred.extend(prep_ops(b + 1))
                emit_main(b)  # drains deferred (prep of b+1 / tail of b-1)
                deferred.extend(tail_ops(b))
            while deferred:
                deferred.pop(0)()

    nc.compile()
    return nc


def _get_nc():
    import os

    batches = int(os.environ.get("K_BATCHES", str(BPC)))
    key = ("nc", batches)
    if key not in _CACHE:
        _CACHE[key] = _build_nc(batches=batches)
    return _CACHE[key]


def _enable_tracing():
    """Shim antenv.axon_hooks (absent in this container) so
    run_bass_kernel_spmd(trace=True) can capture NTFF profiles through
    the axon .so, and neutralize the S3 artifact upload."""
    if _CACHE.get("trace_shim"):
        return
    import types

    import antenv

    if not hasattr(antenv, "axon_hooks"):
        mod = types.ModuleType("antenv.axon_hooks")
        mod._hook = None

        def set_axon_ntff_profile_hook(h):
            mod._hook = h

        def get_axon_ntff_profile_hook():
            return mod._hook

        mod.set_axon_ntff_profile_hook = set_axon_ntff_profile_hook
        mod.get_axon_ntff_profile_hook = get_axon_ntff_profile_hook
        sys.modules["antenv.axon_hooks"] = mod
        antenv.axon_hooks = mod

    from antenv.axon_hooks import get_axon_ntff_profile_hook, set_axon_ntff_profile_hook

    if get_axon_ntff_profile_hook() is None:
        if "/root/.axon_site" not in sys.path:
            sys.path.insert(0, "/root/.axon_site")
        from trn_agent_boot.trn_boot import _ntff_profile_via_ctypes

        set_axon_ntff_profile_hook(
            _ntff_profile_via_ctypes("/opt/axon/libaxon_pjrt.so")
        )

    import concourse.bass_utils as bu

    bu.upload_artifacts = lambda tmpdir: f"local://{tmpdir}"
    _CACHE["trace_shim"] = True


def _in_maps(inputs):
    x = np.ascontiguousarray(np.asarray(inputs["input"], dtype=np.float32))
    m = np.ascontiguousarray(np.asarray(inputs["mask"], dtype=np.float32))
    assert x.shape == (B, L, D) and m.shape == (B, L)
    return [
        {
            "input": np.ascontiguousarray(x[c * BPC : (c + 1) * BPC]),
            "mask": np.ascontiguousarray(m[c * BPC : (c + 1) * BPC]),
        }
        for c in range(NCORES)
    ]


def _run(inputs, trace=False, **kw):
    from concourse.bass_utils import run_bass_kernel_spmd

    if trace:
        _enable_tracing()
    nc = _get_nc()
    res = run_bass_kernel_spmd(
        nc, _in_maps(inputs), core_ids=list(range(NCORES)), trace=trace, **kw
    )
    outs = np.stack([res.results[c]["out"] for c in range(NCORES)])  # [8, BPC, D]
    out_full = outs.reshape(B, 1, D).astype(np.float32)
    attn_mean = np.full((B, L), 1.0 / L, dtype=np.float32)
    return (out_full, attn_mean), res


def kernel(**inputs):
    (out_full, attn_mean), _ = _run(inputs, trace=False)
    return (out_full, attn_mean)
